# revision 9
# baseline (speedup 1.0000x reference)
"""Contrastive loss (NCE softmax over a similarity square) on 8 Trainium2 cores.

Math (B=8192, D=512, T=0.1, r=0.1):
    z   = normalize(emb)                       # row L2
    s   = sum_b emb[b, :]
    v_b = r*s + (1-2r)*emb[b];  pos_b = (z_b . v_b)/||v_b||
    logits row b = [pos_b, raw[b,1:]]/T with raw = z@z.T, diag(raw) tweaks
    loss = mean_b( logsumexp(row_b) - pos_b/T )

Because the row-b fixups cancel, the per-row exp-sum reduces to
    S_b = sum_j exp(raw[b,j]/T) + exp(pos_b/T) - exp(raw[b,b]/T)
and raw[b,b] is recomputed exactly from the quantized z so the subtraction
cancels the in-matrix diagonal term to fp32 rounding.

Sharding: data-parallel over rows. Each core gets the full emb (to build the
all-rows z as matmul rhs) plus its own 1024-row shard, computes its
1024x8192 slice of exp-sums and a partial loss sum; host adds 8 partials.

Per-core pipeline (v2, fp8):
  A. own shard: normalize -> q = fp8e4(8*z) row-major, stage to DRAM,
     XBAR-transpose back as uint16 d-pairs: zTo16[h] = [128, OWN] u16 where
     partition p holds fp8 pair (d=256h+2p, 256h+2p+1) interleaved per byte.
  B. full emb in 4 row-groups of 2048: sq-rowsum (DVE), inv-norm via
     exp(-0.5 ln + ln 8) (ACT, one table set), quantize + s-accumulation on
     GpSimd, q to DRAM, 2 XBAR u16 transposes per group -> zT16[h] columns.
     Matmuls run in DoubleRowSwInterleave fp8 mode (2 k-planes per pass,
     0.5 cyc/col): per (group, m) 2 LDW + 8 matmuls into a [128,2048] psum
     (4 banks), fused exp((10/64)x)+row-sum on ACT, ping-pong 2 psum tiles.
  C. pos path in f32 row-major land (s broadcast via a K=1 fp32 matmul).
  D. S fixup with exact exp diag (from fp8 q), log, partial row-sum via two
     ones-matmuls -> [1,1] output.
"""

import math

import numpy as np

import concourse.bacc as bacc
import concourse.mybir as mybir
import concourse.tile as tile
from concourse.bass_utils import run_bass_kernel_spmd

F32 = mybir.dt.float32
BF16 = mybir.dt.bfloat16
FP8 = mybir.dt.float8e4
U16 = mybir.dt.uint16
AF = mybir.ActivationFunctionType
ALU = mybir.AluOpType
AX = mybir.AxisListType
PM = mybir.MatmulPerfMode

B = 8192
D = 512
N_CORES = 8
OWN = B // N_CORES          # 1024 rows per core
P = 128                     # partitions
NT = B // P                 # 64 full-emb row tiles
NG = 4                      # row groups (transpose pipelining)
TPG = NT // NG              # 16 tiles per group
GR = B // NG                # 2048 rows per group
MT = OWN // P               # 8 own row tiles
NH = 2                      # u16 pair chunks over D (DoubleRow k-tiles)
NSUB = GR // 512            # 512-col matmuls per psum tile
SCALE = 10.0                # 1/TEMPERATURE
RATIO = 0.1
QS = 8.0                    # fp8 pre-scale: q = fp8(QS * z)
QSCALE = SCALE / (QS * QS)  # exp scale applied to q.q psum
LN_QS = float(math.log(QS))


def _body(ctx, tc, out, emb_full, emb_own):
    nc = tc.nc

    pp = ctx.enter_context(tc.tile_pool(name="persist", bufs=1))
    dp = ctx.enter_context(tc.tile_pool(name="dram", bufs=1, space="DRAM"))
    ep = ctx.enter_context(tc.tile_pool(name="ep", bufs=20))
    zp = ctx.enter_context(tc.tile_pool(name="zp", bufs=6))
    scrp = ctx.enter_context(tc.tile_pool(name="scrp", bufs=2))
    up = ctx.enter_context(tc.tile_pool(name="up", bufs=2))
    esp = ctx.enter_context(tc.tile_pool(name="esp", bufs=2))
    psm = ctx.enter_context(tc.tile_pool(name="psm", bufs=2, space="PSUM"))

    # persistent tiles
    zT16 = [pp.tile([P, B], U16, tag=f"zT16_{h}", name=f"zT16_{h}")
            for h in range(NH)]
    zTo16 = [pp.tile([P, OWN], U16, tag=f"zTo16_{h}", name=f"zTo16_{h}")
             for h in range(NH)]
    eo = [pp.tile([P, D], F32, tag=f"eo_{m}", name=f"eo_{m}")
          for m in range(MT)]
    q8o = [pp.tile([P, D], FP8, tag=f"q8o_{m}", name=f"q8o_{m}")
           for m in range(MT)]
    qbo = [pp.tile([P, D], BF16, tag=f"qbo_{m}", name=f"qbo_{m}")
           for m in range(MT)]
    sacc = [pp.tile([P, D], F32, tag=f"sacc_{i}", name=f"sacc_{i}")
            for i in range(4)]
    sqg = pp.tile([P, NT], F32, tag="sqg", name="sqg")
    lng = pp.tile([P, NT], F32, tag="lng", name="lng")
    invq = pp.tile([P, NT], F32, tag="invq", name="invq")
    scols = pp.tile([P, MT * NG], F32, tag="scols", name="scols")
    osq = pp.tile([P, MT], F32, tag="osq", name="osq")
    oln = pp.tile([P, MT], F32, tag="oln", name="oln")
    oinv = pp.tile([P, MT], F32, tag="oinv", name="oinv")
    oinvq = pp.tile([P, MT], F32, tag="oinvq", name="oinvq")
    sdot = pp.tile([P, MT], F32, tag="sdot", name="sdot")
    dexp = pp.tile([P, MT], F32, tag="dexp", name="dexp")
    vsq = pp.tile([P, MT], F32, tag="vsq", name="vsq")
    zvr = pp.tile([P, MT], F32, tag="zvr", name="zvr")
    vln = pp.tile([P, MT], F32, tag="vln", name="vln")
    vninv = pp.tile([P, MT], F32, tag="vninv", name="vninv")
    possim = pp.tile([P, MT], F32, tag="possim", name="possim")
    pos10 = pp.tile([P, MT], F32, tag="pos10", name="pos10")
    epos = pp.tile([P, MT], F32, tag="epos", name="epos")
    stot = pp.tile([P, MT], F32, tag="stot", name="stot")
    sfix = pp.tile([P, MT], F32, tag="sfix", name="sfix")
    lg = pp.tile([P, MT], F32, tag="lg", name="lg")
    loss8 = pp.tile([P, MT], F32, tag="loss8", name="loss8")
    sbc = pp.tile([P, D], F32, tag="sbc", name="sbc")
    s01 = pp.tile([1, D], F32, tag="s01", name="s01")
    lnqs = pp.tile([P, 1], F32, tag="lnqs", name="lnqs")
    ones_row = pp.tile([1, P], F32, tag="ones_row", name="ones_row")
    ones_col = pp.tile([P, 1], F32, tag="ones_col", name="ones_col")
    ones8 = pp.tile([MT, 1], F32, tag="ones8", name="ones8")
    l8 = pp.tile([MT, 1], F32, tag="l8", name="l8")
    res = pp.tile([1, 1], F32, tag="res", name="res")

    qdr = dp.tile([B, D], FP8, tag="qdr", name="qdr")
    qodr = dp.tile([OWN, D], FP8, tag="qodr", name="qodr")

    nc.vector.memset(lnqs, LN_QS)
    nc.vector.memset(ones_row, 1.0)
    nc.vector.memset(ones_col, 1.0)
    nc.vector.memset(ones8, 1.0)
    for i in range(4):
        nc.gpsimd.memset(sacc[i], 0.0)

    def qmm(ps_slice, h, b, cols, own_m):
        """One plain fp8 matmul over k-plane (h, byte b): 512 cols, K=128."""
        lhsT = zTo16[h].bitcast(FP8).rearrange(
            "p (j b) -> p b j", b=2)[:, b, own_m * P:(own_m + 1) * P]
        rhs = zT16[h].bitcast(FP8).rearrange(
            "p (j b) -> p b j", b=2)[:, b, cols[0]:cols[1]]
        nc.tensor.matmul(
            ps_slice, lhsT=lhsT, rhs=rhs,
            start=(h == 0 and b == 0), stop=(h == NH - 1 and b == 1),
            skip_group_check=True)

    # ---- Phase A: own shard -> q8 own + zTo16 ----
    for m in range(MT):
        nc.sync.dma_start(eo[m], emb_own[m * P:(m + 1) * P, :])
    for m in range(MT):
        scr = scrp.tile([P, D], F32, tag="scr", name="scr")
        nc.vector.scalar_tensor_tensor(
            out=scr, in0=eo[m], scalar=1.0, in1=eo[m],
            op0=ALU.mult, op1=ALU.mult, accum_out=osq[:, m:m + 1])
    # inv_norm scales stay in the Ln/Exp table set
    nc.scalar.activation(out=oln, in_=osq, func=AF.Ln)
    nc.scalar.activation(out=oinv, in_=oln, func=AF.Exp, scale=-0.5)
    nc.scalar.activation(out=oinvq, in_=oln, func=AF.Exp, scale=-0.5,
                         bias=lnqs)
    for m in range(MT):
        nc.gpsimd.tensor_scalar_mul(q8o[m], eo[m], oinvq[:, m:m + 1])
        nc.gpsimd.tensor_copy(out=qbo[m], in_=q8o[m])
        nc.sync.dma_start(qodr[m * P:(m + 1) * P, :], q8o[m])
        # exact diagonal: sdot_m = sum_d q^2 (matches PE's fp8 products)
        scr = scrp.tile([P, D], F32, tag="scr", name="scr")
        nc.vector.scalar_tensor_tensor(
            out=scr, in0=qbo[m], scalar=1.0, in1=qbo[m],
            op0=ALU.mult, op1=ALU.mult, accum_out=sdot[:, m:m + 1])
    qodr16 = qodr.bitcast(U16)
    for h in range(NH):
        nc.sync.dma_start_transpose(zTo16[h], qodr16[:, h * P:(h + 1) * P])

    # ---- Phase B: full emb, grouped, software-pipelined emission ----
    qdr16 = qdr.bitcast(U16)

    def emit_norm(g):
        g0, g1 = g * TPG, (g + 1) * TPG
        for t in range(TPG):
            gt = g * TPG + t
            e = ep.tile([P, D], F32, tag="e", name="e")
            nc.sync.dma_start(e, emb_full[gt * P:(gt + 1) * P, :])
            scr = scrp.tile([P, D], F32, tag="scr", name="scr")
            nc.vector.scalar_tensor_tensor(
                out=scr, in0=e, scalar=1.0, in1=e,
                op0=ALU.mult, op1=ALU.mult, accum_out=sqg[:, gt:gt + 1])
            e_tiles.append(e)
        nc.scalar.activation(out=lng[:, g0:g1], in_=sqg[:, g0:g1], func=AF.Ln)
        nc.scalar.activation(out=invq[:, g0:g1], in_=lng[:, g0:g1],
                             func=AF.Exp, scale=-0.5, bias=lnqs)
        for t in range(TPG):
            gt = g * TPG + t
            e = e_tiles[gt]
            q = zp.tile([P, D], FP8, tag="q", name="q")
            nc.gpsimd.tensor_scalar_mul(q, e, invq[:, gt:gt + 1])
            nc.sync.dma_start(qdr[gt * P:(gt + 1) * P, :], q)
            # s accumulation on GpSimd (4 rotating partials, exact f32)
            a = sacc[gt % 4]
            nc.gpsimd.tensor_tensor(out=a, in0=a, in1=e, op=ALU.add)

    def emit_trans(g):
        for h in range(NH):
            nc.sync.dma_start_transpose(
                zT16[h][:, g * GR:(g + 1) * GR],
                qdr16[g * GR:(g + 1) * GR, h * P:(h + 1) * P])

    def emit_main(g):
        for m in range(MT):
            ps = psm.tile([P, GR], F32, tag="ps", name="ps")
            for h in range(NH):
                for b in range(2):
                    for sub in range(NSUB):
                        c0 = g * GR + sub * 512
                        qmm(ps[:, sub * 512:(sub + 1) * 512], h, b,
                            (c0, c0 + 512), m)
            es = esp.tile([P, GR], BF16, tag="es", name="es")
            col = m * NG + g
            nc.scalar.activation(
                out=es, in_=ps, func=AF.Exp, scale=QSCALE,
                accum_out=scols[:, col:col + 1])

    e_tiles = []
    for g in range(NG):
        if g == 0:
            emit_norm(0)
            emit_norm(1)
        elif g + 1 < NG:
            emit_norm(g + 1)
        emit_trans(g)
        emit_main(g)

    # ---- Phase C: positive-pair path ----
    nc.gpsimd.tensor_tensor(out=sacc[0], in0=sacc[0], in1=sacc[1], op=ALU.add)
    nc.gpsimd.tensor_tensor(out=sacc[2], in0=sacc[2], in1=sacc[3], op=ALU.add)
    nc.gpsimd.tensor_tensor(out=sacc[0], in0=sacc[0], in1=sacc[2], op=ALU.add)
    s_psum = psm.tile([1, D], F32, tag="ps", name="ps_s")
    nc.tensor.matmul(s_psum, lhsT=ones_col, rhs=sacc[0], start=True,
                     stop=True)
    nc.vector.tensor_scalar_mul(s01, s_psum, RATIO)
    sb_psum = psm.tile([P, D], F32, tag="ps", name="ps_sbc")
    nc.tensor.matmul(sb_psum, lhsT=ones_row, rhs=s01, start=True, stop=True)
    nc.vector.tensor_copy(out=sbc, in_=sb_psum)
    for m in range(MT):
        u = up.tile([P, D], F32, tag="u", name="u")
        nc.vector.scalar_tensor_tensor(
            out=u, in0=eo[m], scalar=1.0 - 2.0 * RATIO, in1=sbc,
            op0=ALU.mult, op1=ALU.add)
        scr = scrp.tile([P, D], F32, tag="scr", name="scr")
        nc.vector.scalar_tensor_tensor(
            out=scr, in0=u, scalar=1.0, in1=u,
            op0=ALU.mult, op1=ALU.mult, accum_out=vsq[:, m:m + 1])
        scr2 = scrp.tile([P, D], F32, tag="scr", name="scr")
        nc.vector.scalar_tensor_tensor(
            out=scr2, in0=eo[m], scalar=1.0, in1=u,
            op0=ALU.mult, op1=ALU.mult, accum_out=zvr[:, m:m + 1])
    nc.scalar.activation(out=vln, in_=vsq, func=AF.Ln)
    nc.scalar.activation(out=vninv, in_=vln, func=AF.Exp, scale=-0.5)
    # pos = (e.u) * inv_norm_e * inv_norm_v
    nc.vector.tensor_mul(possim, zvr, vninv)
    nc.vector.tensor_mul(possim, possim, oinv)
    nc.vector.tensor_scalar_mul(pos10, possim, SCALE)
    nc.scalar.activation(out=epos, in_=pos10, func=AF.Exp)

    # ---- Phase D: finale ----
    nc.scalar.activation(out=dexp, in_=sdot, func=AF.Exp, scale=QSCALE)
    nc.vector.tensor_reduce(
        stot, scols.rearrange("p (m r) -> p m r", r=NG), axis=AX.X,
        op=ALU.add)
    nc.vector.tensor_sub(sfix, stot, dexp)
    nc.vector.tensor_add(sfix, sfix, epos)
    nc.scalar.activation(out=lg, in_=sfix, func=AF.Ln)
    nc.vector.tensor_sub(loss8, lg, pos10)
    f1 = psm.tile([MT, 1], F32, tag="ps", name="ps_f1")
    nc.tensor.matmul(f1, lhsT=loss8, rhs=ones_col, start=True, stop=True)
    nc.vector.tensor_copy(out=l8, in_=f1)
    f2 = psm.tile([1, 1], F32, tag="ps", name="ps_f2")
    nc.tensor.matmul(f2, lhsT=l8, rhs=ones8, start=True, stop=True)
    nc.vector.tensor_copy(out=res, in_=f2)
    nc.sync.dma_start(out, res)


_NC_CACHE = None


def _build():
    global _NC_CACHE
    if _NC_CACHE is not None:
        return _NC_CACHE
    nc = bacc.Bacc(
        "TRN2",
        target_bir_lowering=False,
        debug=False,
        enable_asserts=False,
        num_devices=N_CORES,
    )
    emb_full = nc.dram_tensor("emb_full", [B, D], F32, kind="ExternalInput").ap()
    emb_own = nc.dram_tensor("emb_own", [OWN, D], F32, kind="ExternalInput").ap()
    out = nc.dram_tensor("out", [1, 1], F32, kind="ExternalOutput").ap()
    from contextlib import ExitStack

    with tile.TileContext(nc) as tc, ExitStack() as ctx:
        _body(ctx, tc, out, emb_full, emb_own)
    nc.compile()
    _NC_CACHE = nc
    return nc


def run(emb: np.ndarray, trace: bool = False):
    """Run the SPMD kernel; returns (loss, BassKernelResults)."""
    emb = np.ascontiguousarray(np.asarray(emb, dtype=np.float32))
    assert emb.shape == (B, D)
    nc = _build()
    in_maps = [
        {
            "emb_full": emb,
            "emb_own": emb[c * OWN:(c + 1) * OWN],
        }
        for c in range(N_CORES)
    ]
    results = run_bass_kernel_spmd(
        nc, in_maps, core_ids=list(range(N_CORES)), trace=trace)
    total = 0.0
    for c in range(N_CORES):
        total += float(results.results[c]["out"][0, 0])
    loss = np.float32(total / B)
    return loss, results


def kernel(emb: np.ndarray) -> np.ndarray:
    loss, _ = run(emb, trace=False)
    return loss


if __name__ == "__main__":
    rng = np.random.default_rng(0)
    x = rng.standard_normal((B, D), dtype=np.float32)
    print("loss:", kernel(x))


# revision 13
# speedup vs baseline: 2.7800x; 2.7800x over previous
"""Contrastive loss (NCE softmax over a similarity square) on 8 Trainium2 cores.

Math (B=8192, D=512, T=0.1, r=0.1):
    z   = normalize(emb)                       # row L2
    s   = sum_b emb[b, :]
    v_b = r*s + (1-2r)*emb[b];  pos_b = (z_b . v_b)/||v_b||
    logits row b = [pos_b, raw[b,1:]]/T with raw = z@z.T, diag(raw) tweaks
    loss = mean_b( logsumexp(row_b) - pos_b/T )

Because the row-b fixups cancel, the per-row exp-sum reduces to
    S_b = sum_j exp(raw[b,j]/T) + exp(pos_b/T) - exp(raw[b,b]/T)
and raw[b,b] is recomputed exactly from the quantized z so the subtraction
cancels the in-matrix diagonal term to fp32 rounding.

Sharding: data-parallel over rows. Each core gets the full emb (to build the
all-rows z as matmul rhs) plus its own 1024-row shard, computes its
1024x8192 slice of exp-sums and a partial loss sum; host adds 8 partials.

Per-core pipeline (v2, fp8):
  A. own shard: normalize -> q = fp8e4(8*z) row-major, stage to DRAM,
     XBAR-transpose back as uint16 d-pairs: zTo16[h] = [128, OWN] u16 where
     partition p holds fp8 pair (d=256h+2p, 256h+2p+1) interleaved per byte.
  B. full emb in 4 row-groups of 2048: sq-rowsum (DVE), inv-norm via
     exp(-0.5 ln + ln 8) (ACT, one table set), quantize + s-accumulation on
     GpSimd, q to DRAM, 2 XBAR u16 transposes per group -> zT16[h] columns.
     Matmuls run in DoubleRowSwInterleave fp8 mode (2 k-planes per pass,
     0.5 cyc/col): per (group, m) 2 LDW + 8 matmuls into a [128,2048] psum
     (4 banks), fused exp((10/64)x)+row-sum on ACT, ping-pong 2 psum tiles.
  C. pos path in f32 row-major land (s broadcast via a K=1 fp32 matmul).
  D. S fixup with exact exp diag (from fp8 q), log, partial row-sum via two
     ones-matmuls -> [1,1] output.
"""

import math

import numpy as np

import concourse.bacc as bacc
import concourse.mybir as mybir
import concourse.tile as tile
from concourse.bass_utils import run_bass_kernel_spmd

F32 = mybir.dt.float32
BF16 = mybir.dt.bfloat16
FP8 = mybir.dt.float8e4
U16 = mybir.dt.uint16
AF = mybir.ActivationFunctionType
ALU = mybir.AluOpType
AX = mybir.AxisListType
PM = mybir.MatmulPerfMode

B = 8192
D = 512
N_CORES = 8
OWN = B // N_CORES          # 1024 rows per core
P = 128                     # partitions
NT = B // P                 # 64 full-emb row tiles
NG = 4                      # row groups (transpose pipelining)
TPG = NT // NG              # 16 tiles per group
GR = B // NG                # 2048 rows per group
MT = OWN // P               # 8 own row tiles
NH = 2                      # u16 pair chunks over D (DoubleRow k-tiles)
NSUB = GR // 512            # 512-col matmuls per psum tile
SCALE = 10.0                # 1/TEMPERATURE
RATIO = 0.1
QS = 8.0                    # fp8 pre-scale: q = fp8(QS * z)
QSCALE = SCALE / (QS * QS)  # exp scale applied to q.q psum
LN_QS = float(math.log(QS))


def _body(ctx, tc, out, emb_full, emb_own):
    nc = tc.nc

    pp = ctx.enter_context(tc.tile_pool(name="persist", bufs=1))
    dp = ctx.enter_context(tc.tile_pool(name="dram", bufs=1, space="DRAM"))
    ep = ctx.enter_context(tc.tile_pool(name="ep", bufs=20))
    zp = ctx.enter_context(tc.tile_pool(name="zp", bufs=6))
    scrp = ctx.enter_context(tc.tile_pool(name="scrp", bufs=2))
    up = ctx.enter_context(tc.tile_pool(name="up", bufs=2))
    esp = ctx.enter_context(tc.tile_pool(name="esp", bufs=2))
    psm = ctx.enter_context(tc.tile_pool(name="psm", bufs=2, space="PSUM"))

    # persistent tiles
    zT16 = [pp.tile([P, B], U16, tag=f"zT16_{h}", name=f"zT16_{h}")
            for h in range(NH)]
    zTo16 = [pp.tile([P, OWN], U16, tag=f"zTo16_{h}", name=f"zTo16_{h}")
             for h in range(NH)]
    eo = [pp.tile([P, D], F32, tag=f"eo_{m}", name=f"eo_{m}")
          for m in range(MT)]
    q8o = [pp.tile([P, D], FP8, tag=f"q8o_{m}", name=f"q8o_{m}")
           for m in range(MT)]
    qbo = [pp.tile([P, D], BF16, tag=f"qbo_{m}", name=f"qbo_{m}")
           for m in range(MT)]
    sacc = [pp.tile([P, D], F32, tag=f"sacc_{i}", name=f"sacc_{i}")
            for i in range(4)]
    sqg = pp.tile([P, NT], F32, tag="sqg", name="sqg")
    lng = pp.tile([P, NT], F32, tag="lng", name="lng")
    invq = pp.tile([P, NT], F32, tag="invq", name="invq")
    scols = pp.tile([P, MT * NG], F32, tag="scols", name="scols")
    osq = pp.tile([P, MT], F32, tag="osq", name="osq")
    oln = pp.tile([P, MT], F32, tag="oln", name="oln")
    oinv = pp.tile([P, MT], F32, tag="oinv", name="oinv")
    oinvq = pp.tile([P, MT], F32, tag="oinvq", name="oinvq")
    sdot = pp.tile([P, MT], F32, tag="sdot", name="sdot")
    dexp = pp.tile([P, MT], F32, tag="dexp", name="dexp")
    vsq = pp.tile([P, MT], F32, tag="vsq", name="vsq")
    zvr = pp.tile([P, MT], F32, tag="zvr", name="zvr")
    vln = pp.tile([P, MT], F32, tag="vln", name="vln")
    vninv = pp.tile([P, MT], F32, tag="vninv", name="vninv")
    possim = pp.tile([P, MT], F32, tag="possim", name="possim")
    pos10 = pp.tile([P, MT], F32, tag="pos10", name="pos10")
    epos = pp.tile([P, MT], F32, tag="epos", name="epos")
    stot = pp.tile([P, MT], F32, tag="stot", name="stot")
    sfix = pp.tile([P, MT], F32, tag="sfix", name="sfix")
    lg = pp.tile([P, MT], F32, tag="lg", name="lg")
    loss8 = pp.tile([P, MT], F32, tag="loss8", name="loss8")
    sbc = pp.tile([P, D], F32, tag="sbc", name="sbc")
    s01 = pp.tile([1, D], F32, tag="s01", name="s01")
    lnqs = pp.tile([P, 1], F32, tag="lnqs", name="lnqs")
    ones_row = pp.tile([1, P], F32, tag="ones_row", name="ones_row")
    ones_col = pp.tile([P, 1], F32, tag="ones_col", name="ones_col")
    ones8 = pp.tile([MT, 1], F32, tag="ones8", name="ones8")
    l8 = pp.tile([MT, 1], F32, tag="l8", name="l8")
    res = pp.tile([1, 1], F32, tag="res", name="res")

    qdr = dp.tile([B, D], FP8, tag="qdr", name="qdr")
    qodr = dp.tile([OWN, D], FP8, tag="qodr", name="qodr")

    nc.vector.memset(lnqs, LN_QS)
    nc.vector.memset(ones_row, 1.0)
    nc.vector.memset(ones_col, 1.0)
    nc.vector.memset(ones8, 1.0)
    for i in range(4):
        nc.vector.memset(sacc[i], 0.0)

    def qmm(ps_slice, h, b, cols, own_m):
        """One plain fp8 matmul over k-plane (h, byte b): 512 cols, K=128."""
        lhsT = zTo16[h].bitcast(FP8).rearrange(
            "p (j b) -> p b j", b=2)[:, b, own_m * P:(own_m + 1) * P]
        rhs = zT16[h].bitcast(FP8).rearrange(
            "p (j b) -> p b j", b=2)[:, b, cols[0]:cols[1]]
        nc.tensor.matmul(
            ps_slice, lhsT=lhsT, rhs=rhs,
            start=(h == 0 and b == 0), stop=(h == NH - 1 and b == 1),
            skip_group_check=True)

    # ---- Phase A: own shard -> q8 own + zTo16 ----
    for m in range(MT):
        nc.sync.dma_start(eo[m], emb_own[m * P:(m + 1) * P, :])
    for m in range(MT):
        scr = scrp.tile([P, D], F32, tag="scr", name="scr")
        nc.vector.scalar_tensor_tensor(
            out=scr, in0=eo[m], scalar=1.0, in1=eo[m],
            op0=ALU.mult, op1=ALU.mult, accum_out=osq[:, m:m + 1])
    # inv_norm scales stay in the Ln/Exp table set
    nc.scalar.activation(out=oln, in_=osq, func=AF.Ln)
    nc.scalar.activation(out=oinv, in_=oln, func=AF.Exp, scale=-0.5)
    nc.scalar.activation(out=oinvq, in_=oln, func=AF.Exp, scale=-0.5,
                         bias=lnqs)
    for m in range(MT):
        nc.vector.tensor_scalar_mul(q8o[m], eo[m], oinvq[:, m:m + 1])
        nc.gpsimd.tensor_copy(out=qbo[m], in_=q8o[m])
        nc.sync.dma_start(qodr[m * P:(m + 1) * P, :], q8o[m])
        # exact diagonal: sdot_m = sum_d q^2 (matches PE's fp8 products)
        scr = scrp.tile([P, D], F32, tag="scr", name="scr")
        nc.vector.scalar_tensor_tensor(
            out=scr, in0=qbo[m], scalar=1.0, in1=qbo[m],
            op0=ALU.mult, op1=ALU.mult, accum_out=sdot[:, m:m + 1])
    qodr16 = qodr.bitcast(U16)
    for h in range(NH):
        nc.sync.dma_start_transpose(zTo16[h], qodr16[:, h * P:(h + 1) * P])

    # ---- Phase B: full emb, grouped, software-pipelined emission ----
    qdr16 = qdr.bitcast(U16)

    def emit_norm(g):
        g0, g1 = g * TPG, (g + 1) * TPG
        for t in range(TPG):
            gt = g * TPG + t
            e = ep.tile([P, D], F32, tag="e", name="e")
            nc.sync.dma_start(e, emb_full[gt * P:(gt + 1) * P, :])
            scr = scrp.tile([P, D], F32, tag="scr", name="scr")
            nc.vector.scalar_tensor_tensor(
                out=scr, in0=e, scalar=1.0, in1=e,
                op0=ALU.mult, op1=ALU.mult, accum_out=sqg[:, gt:gt + 1])
            e_tiles.append(e)
        nc.scalar.activation(out=lng[:, g0:g1], in_=sqg[:, g0:g1], func=AF.Ln)
        nc.scalar.activation(out=invq[:, g0:g1], in_=lng[:, g0:g1],
                             func=AF.Exp, scale=-0.5, bias=lnqs)
        for t in range(TPG):
            gt = g * TPG + t
            e = e_tiles[gt]
            q = zp.tile([P, D], FP8, tag="q", name="q")
            nc.vector.tensor_scalar_mul(q, e, invq[:, gt:gt + 1])
            nc.sync.dma_start(qdr[gt * P:(gt + 1) * P, :], q)
            # s accumulation on DVE (4 rotating partials, exact f32)
            a = sacc[gt % 4]
            nc.vector.tensor_tensor(out=a, in0=a, in1=e, op=ALU.add)

    def emit_trans(g):
        for h in range(NH):
            nc.sync.dma_start_transpose(
                zT16[h][:, g * GR:(g + 1) * GR],
                qdr16[g * GR:(g + 1) * GR, h * P:(h + 1) * P])

    def emit_main(g):
        for m in range(MT):
            ps = psm.tile([P, GR], F32, tag="ps", name="ps")
            for h in range(NH):
                for b in range(2):
                    for sub in range(NSUB):
                        c0 = g * GR + sub * 512
                        qmm(ps[:, sub * 512:(sub + 1) * 512], h, b,
                            (c0, c0 + 512), m)
            es = esp.tile([P, GR], BF16, tag="es", name="es")
            col = m * NG + g
            nc.scalar.activation(
                out=es, in_=ps, func=AF.Exp, scale=QSCALE,
                accum_out=scols[:, col:col + 1])

    e_tiles = []
    for g in range(NG):
        if g == 0:
            emit_norm(0)
            emit_norm(1)
        elif g + 1 < NG:
            emit_norm(g + 1)
        emit_trans(g)
        emit_main(g)

    # ---- Phase C: positive-pair path ----
    nc.vector.tensor_tensor(out=sacc[0], in0=sacc[0], in1=sacc[1], op=ALU.add)
    nc.vector.tensor_tensor(out=sacc[2], in0=sacc[2], in1=sacc[3], op=ALU.add)
    nc.vector.tensor_tensor(out=sacc[0], in0=sacc[0], in1=sacc[2], op=ALU.add)
    s_psum = psm.tile([1, D], F32, tag="ps", name="ps_s")
    nc.tensor.matmul(s_psum, lhsT=ones_col, rhs=sacc[0], start=True,
                     stop=True)
    nc.vector.tensor_scalar_mul(s01, s_psum, RATIO)
    sb_psum = psm.tile([P, D], F32, tag="ps", name="ps_sbc")
    nc.tensor.matmul(sb_psum, lhsT=ones_row, rhs=s01, start=True, stop=True)
    nc.vector.tensor_copy(out=sbc, in_=sb_psum)
    for m in range(MT):
        u = up.tile([P, D], F32, tag="u", name="u")
        nc.vector.scalar_tensor_tensor(
            out=u, in0=eo[m], scalar=1.0 - 2.0 * RATIO, in1=sbc,
            op0=ALU.mult, op1=ALU.add)
        scr = scrp.tile([P, D], F32, tag="scr", name="scr")
        nc.vector.scalar_tensor_tensor(
            out=scr, in0=u, scalar=1.0, in1=u,
            op0=ALU.mult, op1=ALU.mult, accum_out=vsq[:, m:m + 1])
        scr2 = scrp.tile([P, D], F32, tag="scr", name="scr")
        nc.vector.scalar_tensor_tensor(
            out=scr2, in0=eo[m], scalar=1.0, in1=u,
            op0=ALU.mult, op1=ALU.mult, accum_out=zvr[:, m:m + 1])
    nc.scalar.activation(out=vln, in_=vsq, func=AF.Ln)
    nc.scalar.activation(out=vninv, in_=vln, func=AF.Exp, scale=-0.5)
    # pos = (e.u) * inv_norm_e * inv_norm_v
    nc.vector.tensor_mul(possim, zvr, vninv)
    nc.vector.tensor_mul(possim, possim, oinv)
    nc.vector.tensor_scalar_mul(pos10, possim, SCALE)
    nc.scalar.activation(out=epos, in_=pos10, func=AF.Exp)

    # ---- Phase D: finale ----
    nc.scalar.activation(out=dexp, in_=sdot, func=AF.Exp, scale=QSCALE)
    nc.vector.tensor_reduce(
        stot, scols.rearrange("p (m r) -> p m r", r=NG), axis=AX.X,
        op=ALU.add)
    nc.vector.tensor_sub(sfix, stot, dexp)
    nc.vector.tensor_add(sfix, sfix, epos)
    nc.scalar.activation(out=lg, in_=sfix, func=AF.Ln)
    nc.vector.tensor_sub(loss8, lg, pos10)
    f1 = psm.tile([MT, 1], F32, tag="ps", name="ps_f1")
    nc.tensor.matmul(f1, lhsT=loss8, rhs=ones_col, start=True, stop=True)
    nc.vector.tensor_copy(out=l8, in_=f1)
    f2 = psm.tile([1, 1], F32, tag="ps", name="ps_f2")
    nc.tensor.matmul(f2, lhsT=l8, rhs=ones8, start=True, stop=True)
    nc.vector.tensor_copy(out=res, in_=f2)
    nc.sync.dma_start(out, res)


_NC_CACHE = None


def _build():
    global _NC_CACHE
    if _NC_CACHE is not None:
        return _NC_CACHE
    nc = bacc.Bacc(
        "TRN2",
        target_bir_lowering=False,
        debug=False,
        enable_asserts=False,
        num_devices=N_CORES,
    )
    emb_full = nc.dram_tensor("emb_full", [B, D], F32, kind="ExternalInput").ap()
    emb_own = nc.dram_tensor("emb_own", [OWN, D], F32, kind="ExternalInput").ap()
    out = nc.dram_tensor("out", [1, 1], F32, kind="ExternalOutput").ap()
    from contextlib import ExitStack

    with tile.TileContext(nc) as tc, ExitStack() as ctx:
        _body(ctx, tc, out, emb_full, emb_own)
    nc.compile()
    _NC_CACHE = nc
    return nc


def run(emb: np.ndarray, trace: bool = False):
    """Run the SPMD kernel; returns (loss, BassKernelResults)."""
    emb = np.ascontiguousarray(np.asarray(emb, dtype=np.float32))
    assert emb.shape == (B, D)
    nc = _build()
    in_maps = [
        {
            "emb_full": emb,
            "emb_own": emb[c * OWN:(c + 1) * OWN],
        }
        for c in range(N_CORES)
    ]
    results = run_bass_kernel_spmd(
        nc, in_maps, core_ids=list(range(N_CORES)), trace=trace)
    total = 0.0
    for c in range(N_CORES):
        total += float(results.results[c]["out"][0, 0])
    loss = np.float32(total / B)
    return loss, results


def kernel(emb: np.ndarray) -> np.ndarray:
    loss, _ = run(emb, trace=False)
    return loss


if __name__ == "__main__":
    rng = np.random.default_rng(0)
    x = rng.standard_normal((B, D), dtype=np.float32)
    print("loss:", kernel(x))


# revision 15
# speedup vs baseline: 3.3586x; 1.2081x over previous
"""Contrastive loss (NCE softmax over a similarity square) on 8 Trainium2 cores.

Math (B=8192, D=512, T=0.1, r=0.1):
    z   = normalize(emb)                       # row L2
    s   = sum_b emb[b, :]
    v_b = r*s + (1-2r)*emb[b];  pos_b = (z_b . v_b)/||v_b||
    logits row b = [pos_b, raw[b,1:]]/T with raw = z@z.T, diag(raw) tweaks
    loss = mean_b( logsumexp(row_b) - pos_b/T )

Because the row-b fixups cancel, the per-row exp-sum reduces to
    S_b = sum_j exp(raw[b,j]/T) + exp(pos_b/T) - exp(raw[b,b]/T)
and raw[b,b] is recomputed exactly from the quantized z so the subtraction
cancels the in-matrix diagonal term to fp32 rounding.

Sharding: data-parallel over rows. Each core gets the full emb (to build the
all-rows z as matmul rhs) plus its own 1024-row shard, computes its
1024x8192 slice of exp-sums and a partial loss sum; host adds 8 partials.

Per-core pipeline (v2, fp8):
  A. own shard: normalize -> q = fp8e4(8*z) row-major, stage to DRAM,
     XBAR-transpose back as uint16 d-pairs: zTo16[h] = [128, OWN] u16 where
     partition p holds fp8 pair (d=256h+2p, 256h+2p+1) interleaved per byte.
  B. full emb in 4 row-groups of 2048: sq-rowsum (DVE), inv-norm via
     exp(-0.5 ln + ln 8) (ACT, one table set), quantize + s-accumulation on
     GpSimd, q to DRAM, 2 XBAR u16 transposes per group -> zT16[h] columns.
     Matmuls run in DoubleRowSwInterleave fp8 mode (2 k-planes per pass,
     0.5 cyc/col): per (group, m) 2 LDW + 8 matmuls into a [128,2048] psum
     (4 banks), fused exp((10/64)x)+row-sum on ACT, ping-pong 2 psum tiles.
  C. pos path in f32 row-major land (s broadcast via a K=1 fp32 matmul).
  D. S fixup with exact exp diag (from fp8 q), log, partial row-sum via two
     ones-matmuls -> [1,1] output.
"""

import math

import numpy as np

import concourse.bacc as bacc
import concourse.mybir as mybir
import concourse.tile as tile
from concourse.bass_utils import run_bass_kernel_spmd

F32 = mybir.dt.float32
BF16 = mybir.dt.bfloat16
FP8 = mybir.dt.float8e4
U16 = mybir.dt.uint16
AF = mybir.ActivationFunctionType
ALU = mybir.AluOpType
AX = mybir.AxisListType
PM = mybir.MatmulPerfMode

B = 8192
D = 512
N_CORES = 8
OWN = B // N_CORES          # 1024 rows per core
P = 128                     # partitions
NT = B // P                 # 64 full-emb row tiles
NG = 4                      # row groups (transpose pipelining)
TPG = NT // NG              # 16 tiles per group
GR = B // NG                # 2048 rows per group
MT = OWN // P               # 8 own row tiles
NH = 2                      # u16 pair chunks over D (DoubleRow k-tiles)
NSUB = GR // 512            # 512-col matmuls per psum tile
SCALE = 10.0                # 1/TEMPERATURE
RATIO = 0.1
QS = 8.0                    # fp8 pre-scale: q = fp8(QS * z)
QSCALE = SCALE / (QS * QS)  # exp scale applied to q.q psum
LN_QS = float(math.log(QS))


def _body(ctx, tc, out, emb_full, emb_own):
    nc = tc.nc

    pp = ctx.enter_context(tc.tile_pool(name="persist", bufs=1))
    dp = ctx.enter_context(tc.tile_pool(name="dram", bufs=1, space="DRAM"))
    ep = ctx.enter_context(tc.tile_pool(name="ep", bufs=20))
    zp = ctx.enter_context(tc.tile_pool(name="zp", bufs=6))
    scrp = ctx.enter_context(tc.tile_pool(name="scrp", bufs=2))
    up = ctx.enter_context(tc.tile_pool(name="up", bufs=2))
    esp = ctx.enter_context(tc.tile_pool(name="esp", bufs=2))
    psm = ctx.enter_context(tc.tile_pool(name="psm", bufs=2, space="PSUM"))

    # persistent tiles
    zT16 = [pp.tile([P, B], U16, tag=f"zT16_{h}", name=f"zT16_{h}")
            for h in range(NH)]
    zTo16 = [pp.tile([P, OWN], U16, tag=f"zTo16_{h}", name=f"zTo16_{h}")
             for h in range(NH)]
    eo = [pp.tile([P, D], F32, tag=f"eo_{m}", name=f"eo_{m}")
          for m in range(MT)]
    q8o = [pp.tile([P, D], FP8, tag=f"q8o_{m}", name=f"q8o_{m}")
           for m in range(MT)]
    qbo = [pp.tile([P, D], BF16, tag=f"qbo_{m}", name=f"qbo_{m}")
           for m in range(MT)]
    sacc = [pp.tile([P, D], F32, tag=f"sacc_{i}", name=f"sacc_{i}")
            for i in range(4)]
    sqg = pp.tile([P, NT], F32, tag="sqg", name="sqg")
    lng = pp.tile([P, NT], F32, tag="lng", name="lng")
    invq = pp.tile([P, NT], F32, tag="invq", name="invq")
    scols = pp.tile([P, MT * NG], F32, tag="scols", name="scols")
    osq = pp.tile([P, MT], F32, tag="osq", name="osq")
    oln = pp.tile([P, MT], F32, tag="oln", name="oln")
    oinv = pp.tile([P, MT], F32, tag="oinv", name="oinv")
    oinvq = pp.tile([P, MT], F32, tag="oinvq", name="oinvq")
    sdot = pp.tile([P, MT], F32, tag="sdot", name="sdot")
    dexp = pp.tile([P, MT], F32, tag="dexp", name="dexp")
    vsq = pp.tile([P, MT], F32, tag="vsq", name="vsq")
    zvr = pp.tile([P, MT], F32, tag="zvr", name="zvr")
    vln = pp.tile([P, MT], F32, tag="vln", name="vln")
    vninv = pp.tile([P, MT], F32, tag="vninv", name="vninv")
    possim = pp.tile([P, MT], F32, tag="possim", name="possim")
    pos10 = pp.tile([P, MT], F32, tag="pos10", name="pos10")
    epos = pp.tile([P, MT], F32, tag="epos", name="epos")
    stot = pp.tile([P, MT], F32, tag="stot", name="stot")
    sfix = pp.tile([P, MT], F32, tag="sfix", name="sfix")
    lg = pp.tile([P, MT], F32, tag="lg", name="lg")
    loss8 = pp.tile([P, MT], F32, tag="loss8", name="loss8")
    sbc = pp.tile([P, D], F32, tag="sbc", name="sbc")
    s01 = pp.tile([1, D], F32, tag="s01", name="s01")
    lnqs = pp.tile([P, 1], F32, tag="lnqs", name="lnqs")
    ones_row = pp.tile([1, P], F32, tag="ones_row", name="ones_row")
    ones_col = pp.tile([P, 1], F32, tag="ones_col", name="ones_col")
    ones8 = pp.tile([MT, 1], F32, tag="ones8", name="ones8")
    l8 = pp.tile([MT, 1], F32, tag="l8", name="l8")
    res = pp.tile([1, 1], F32, tag="res", name="res")

    qdr = dp.tile([B, D], FP8, tag="qdr", name="qdr")
    qodr = dp.tile([OWN, D], FP8, tag="qodr", name="qodr")

    nc.vector.memset(lnqs, LN_QS)
    nc.vector.memset(ones_row, 1.0)
    nc.vector.memset(ones_col, 1.0)
    nc.vector.memset(ones8, 1.0)
    for i in range(4):
        nc.vector.memset(sacc[i], 0.0)

    def qmm(ps_slice, h, b, cols, own_m):
        """One plain fp8 matmul over k-plane (h, byte b): 512 cols, K=128."""
        lhsT = zTo16[h].bitcast(FP8).rearrange(
            "p (j b) -> p b j", b=2)[:, b, own_m * P:(own_m + 1) * P]
        rhs = zT16[h].bitcast(FP8).rearrange(
            "p (j b) -> p b j", b=2)[:, b, cols[0]:cols[1]]
        nc.tensor.matmul(
            ps_slice, lhsT=lhsT, rhs=rhs,
            start=(h == 0 and b == 0), stop=(h == NH - 1 and b == 1),
            skip_group_check=True)

    # ---- Phase A: own shard -> q8 own + zTo16 ----
    for m in range(MT):
        nc.sync.dma_start(eo[m], emb_own[m * P:(m + 1) * P, :])
    for m in range(MT):
        scr = scrp.tile([P, D], F32, tag="scr", name="scr")
        nc.vector.scalar_tensor_tensor(
            out=scr, in0=eo[m], scalar=1.0, in1=eo[m],
            op0=ALU.mult, op1=ALU.mult, accum_out=osq[:, m:m + 1])
    # inv_norm scales stay in the Ln/Exp table set
    nc.scalar.activation(out=oln, in_=osq, func=AF.Ln)
    nc.scalar.activation(out=oinv, in_=oln, func=AF.Exp, scale=-0.5)
    nc.scalar.activation(out=oinvq, in_=oln, func=AF.Exp, scale=-0.5,
                         bias=lnqs)
    for m in range(MT):
        nc.vector.tensor_scalar_mul(q8o[m], eo[m], oinvq[:, m:m + 1])
        nc.gpsimd.tensor_copy(out=qbo[m], in_=q8o[m])
        nc.sync.dma_start(qodr[m * P:(m + 1) * P, :], q8o[m])
        # exact diagonal: sdot_m = sum_d q^2 (matches PE's fp8 products)
        scr = scrp.tile([P, D], F32, tag="scr", name="scr")
        nc.vector.scalar_tensor_tensor(
            out=scr, in0=qbo[m], scalar=1.0, in1=qbo[m],
            op0=ALU.mult, op1=ALU.mult, accum_out=sdot[:, m:m + 1])
    qodr16 = qodr.bitcast(U16)
    for h in range(NH):
        nc.sync.dma_start_transpose(zTo16[h], qodr16[:, h * P:(h + 1) * P])

    # ---- Phase B: full emb, grouped, software-pipelined emission ----
    qdr16 = qdr.bitcast(U16)

    def emit_norm(g):
        g0, g1 = g * TPG, (g + 1) * TPG
        for t in range(TPG):
            gt = g * TPG + t
            e = ep.tile([P, D], F32, tag="e", name="e")
            nc.sync.dma_start(e, emb_full[gt * P:(gt + 1) * P, :])
            scr = scrp.tile([P, D], F32, tag="scr", name="scr")
            nc.vector.scalar_tensor_tensor(
                out=scr, in0=e, scalar=1.0, in1=e,
                op0=ALU.mult, op1=ALU.mult, accum_out=sqg[:, gt:gt + 1])
            e_tiles.append(e)
        nc.scalar.activation(out=lng[:, g0:g1], in_=sqg[:, g0:g1], func=AF.Ln)
        nc.scalar.activation(out=invq[:, g0:g1], in_=lng[:, g0:g1],
                             func=AF.Exp, scale=-0.5, bias=lnqs)
        for t in range(TPG):
            gt = g * TPG + t
            e = e_tiles[gt]
            q = zp.tile([P, D], FP8, tag="q", name="q")
            nc.vector.tensor_scalar_mul(q, e, invq[:, gt:gt + 1])
            nc.sync.dma_start(qdr[gt * P:(gt + 1) * P, :], q)
            # s accumulation on DVE (4 rotating partials, exact f32)
            a = sacc[gt % 4]
            nc.vector.tensor_tensor(out=a, in0=a, in1=e, op=ALU.add)

    def emit_trans(g):
        for h in range(NH):
            nc.sync.dma_start_transpose(
                zT16[h][:, g * GR:(g + 1) * GR],
                qdr16[g * GR:(g + 1) * GR, h * P:(h + 1) * P])

    def emit_main(g):
        for m in range(MT):
            ps = psm.tile([P, GR], F32, tag="ps", name="ps")
            for h in range(NH):
                for b in range(2):
                    for sub in range(NSUB):
                        c0 = g * GR + sub * 512
                        qmm(ps[:, sub * 512:(sub + 1) * 512], h, b,
                            (c0, c0 + 512), m)
            es = esp.tile([P, GR], BF16, tag="es", name="es")
            col = m * NG + g
            nc.scalar.activation(
                out=es, in_=ps, func=AF.Exp, scale=QSCALE,
                accum_out=scols[:, col:col + 1])

    def emit_posC():
        # ---- Phase C: positive-pair path (overlaps with last main group:
        # depends only on sacc + eo/oinv, not on scols) ----
        nc.vector.tensor_tensor(out=sacc[0], in0=sacc[0], in1=sacc[1],
                                op=ALU.add)
        nc.vector.tensor_tensor(out=sacc[2], in0=sacc[2], in1=sacc[3],
                                op=ALU.add)
        nc.vector.tensor_tensor(out=sacc[0], in0=sacc[0], in1=sacc[2],
                                op=ALU.add)
        s_psum = psm.tile([1, D], F32, tag="ps", name="ps_s")
        nc.tensor.matmul(s_psum, lhsT=ones_col, rhs=sacc[0], start=True,
                         stop=True)
        nc.vector.tensor_scalar_mul(s01, s_psum, RATIO)
        sb_psum = psm.tile([P, D], F32, tag="ps", name="ps_sbc")
        nc.tensor.matmul(sb_psum, lhsT=ones_row, rhs=s01, start=True,
                         stop=True)
        nc.vector.tensor_copy(out=sbc, in_=sb_psum)
        for m in range(MT):
            u = up.tile([P, D], F32, tag="u", name="u")
            nc.vector.scalar_tensor_tensor(
                out=u, in0=eo[m], scalar=1.0 - 2.0 * RATIO, in1=sbc,
                op0=ALU.mult, op1=ALU.add)
            scr = scrp.tile([P, D], F32, tag="scr", name="scr")
            nc.vector.scalar_tensor_tensor(
                out=scr, in0=u, scalar=1.0, in1=u,
                op0=ALU.mult, op1=ALU.mult, accum_out=vsq[:, m:m + 1])
            scr2 = scrp.tile([P, D], F32, tag="scr", name="scr")
            nc.vector.scalar_tensor_tensor(
                out=scr2, in0=eo[m], scalar=1.0, in1=u,
                op0=ALU.mult, op1=ALU.mult, accum_out=zvr[:, m:m + 1])
        nc.scalar.activation(out=vln, in_=vsq, func=AF.Ln)
        nc.scalar.activation(out=vninv, in_=vln, func=AF.Exp, scale=-0.5)
        # pos = (e.u) * inv_norm_e * inv_norm_v
        nc.vector.tensor_mul(possim, zvr, vninv)
        nc.vector.tensor_mul(possim, possim, oinv)
        nc.vector.tensor_scalar_mul(pos10, possim, SCALE)
        nc.scalar.activation(out=epos, in_=pos10, func=AF.Exp)

    e_tiles = []
    for g in range(NG):
        if g == 0:
            emit_norm(0)
            emit_norm(1)
        elif g + 1 < NG:
            emit_norm(g + 1)
        emit_trans(g)
        if g == NG - 1:
            emit_posC()
        emit_main(g)

    # ---- Phase D: finale ----
    nc.scalar.activation(out=dexp, in_=sdot, func=AF.Exp, scale=QSCALE)
    nc.vector.tensor_reduce(
        stot, scols.rearrange("p (m r) -> p m r", r=NG), axis=AX.X,
        op=ALU.add)
    nc.vector.tensor_sub(sfix, stot, dexp)
    nc.vector.tensor_add(sfix, sfix, epos)
    nc.scalar.activation(out=lg, in_=sfix, func=AF.Ln)
    nc.vector.tensor_sub(loss8, lg, pos10)
    f1 = psm.tile([MT, 1], F32, tag="ps", name="ps_f1")
    nc.tensor.matmul(f1, lhsT=loss8, rhs=ones_col, start=True, stop=True)
    nc.vector.tensor_copy(out=l8, in_=f1)
    f2 = psm.tile([1, 1], F32, tag="ps", name="ps_f2")
    nc.tensor.matmul(f2, lhsT=l8, rhs=ones8, start=True, stop=True)
    nc.vector.tensor_copy(out=res, in_=f2)
    nc.sync.dma_start(out, res)


_NC_CACHE = None


def _build():
    global _NC_CACHE
    if _NC_CACHE is not None:
        return _NC_CACHE
    nc = bacc.Bacc(
        "TRN2",
        target_bir_lowering=False,
        debug=False,
        enable_asserts=False,
        num_devices=N_CORES,
    )
    emb_full = nc.dram_tensor("emb_full", [B, D], F32, kind="ExternalInput").ap()
    emb_own = nc.dram_tensor("emb_own", [OWN, D], F32, kind="ExternalInput").ap()
    out = nc.dram_tensor("out", [1, 1], F32, kind="ExternalOutput").ap()
    from contextlib import ExitStack

    with tile.TileContext(nc) as tc, ExitStack() as ctx:
        _body(ctx, tc, out, emb_full, emb_own)
    nc.compile()
    _NC_CACHE = nc
    return nc


def run(emb: np.ndarray, trace: bool = False):
    """Run the SPMD kernel; returns (loss, BassKernelResults)."""
    emb = np.ascontiguousarray(np.asarray(emb, dtype=np.float32))
    assert emb.shape == (B, D)
    nc = _build()
    in_maps = [
        {
            "emb_full": emb,
            "emb_own": emb[c * OWN:(c + 1) * OWN],
        }
        for c in range(N_CORES)
    ]
    results = run_bass_kernel_spmd(
        nc, in_maps, core_ids=list(range(N_CORES)), trace=trace)
    total = 0.0
    for c in range(N_CORES):
        total += float(results.results[c]["out"][0, 0])
    loss = np.float32(total / B)
    return loss, results


def kernel(emb: np.ndarray) -> np.ndarray:
    loss, _ = run(emb, trace=False)
    return loss


if __name__ == "__main__":
    rng = np.random.default_rng(0)
    x = rng.standard_normal((B, D), dtype=np.float32)
    print("loss:", kernel(x))
